# revision 27
# baseline (speedup 1.0000x reference)
"""Trainium2 Bass kernel for nn_Detector_head (SuperPoint-style detector head).

Pipeline per sample: 3x3 conv(256->256)+BN+ReLU -> 1x1 conv(256->65)+BN ->
softmax(65) -> drop dustbin -> pixel_shuffle(8) -> greedy box-NMS -> top-300.

Sharding: pure data parallelism, batch 32 -> 8 cores x 4 samples.
"""

import sys

sys.path.insert(0, "/opt/trn_rl_repo")

import numpy as np

B_PER_CORE = 4
CIN = 256
CMID = 256
COUT = 65
H, W = 60, 80
PIX = H * W  # 4800
HP, WP = H + 2, W + 2  # 62, 82
PPIX = HP * WP  # 5084
GRID = 8
HH, HW_ = H * GRID, W * GRID  # 480, 640
HEAT_N = HH * HW_  # 307200
NTILE = 10  # pixel tiles for conv (480 each)
TILE_PIX = PIX // NTILE  # 480
TROWS = TILE_PIX // W  # 6 rows per tile
EPS = 1e-5

NMS_SIZE = 4.0
IOU_TH = 0.1
MIN_PROB = 0.015
TOP_K = 300
NMS_CAND = 1024

_CACHED = {}


def _last_in_maps_get():
    return _CACHED.get("last_in_maps")


def _build_nc():
    import concourse.bacc as bacc
    import concourse.mybir as mybir
    from concourse.tile import TileContext

    f32 = mybir.dt.float32
    bf16 = mybir.dt.bfloat16
    nc = bacc.Bacc("TRN2", target_bir_lowering=False, debug=False, num_devices=8)

    xh_ext = nc.declare_dram_parameter(
        "x_hi", [B_PER_CORE, CIN, H, W], bf16, isOutput=False
    )
    xl_ext = nc.declare_dram_parameter(
        "x_lo", [B_PER_CORE, CIN, H, W], bf16, isOutput=False
    )
    wah_ext = nc.declare_dram_parameter(
        "waT_hi", [128, 18, CMID], bf16, isOutput=False
    )
    wal_ext = nc.declare_dram_parameter(
        "waT_lo", [128, 18, CMID], bf16, isOutput=False
    )
    wbh_ext = nc.declare_dram_parameter("wbT_hi", [128, 2, COUT], bf16, isOutput=False)
    wbl_ext = nc.declare_dram_parameter("wbT_lo", [128, 2, COUT], bf16, isOutput=False)
    ba_ext = nc.declare_dram_parameter("bias_a", [CMID], f32, isOutput=False)
    bb_ext = nc.declare_dram_parameter("bias_b", [COUT, 1], f32, isOutput=False)
    ones_ext = nc.declare_dram_parameter("ones128", [128, 128], f32, isOutput=False)

    s_dram = nc.dram_tensor("s_scratch", [B_PER_CORE, PIX], f32)
    logits_out = nc.declare_dram_parameter(
        "logits", [B_PER_CORE, COUT, H, W], f32, isOutput=True
    )
    heat_out = nc.declare_dram_parameter(
        "heat", [B_PER_CORE, HH, HW_], f32, isOutput=True
    )

    with TileContext(nc) as tc:
        with (
            tc.tile_pool(name="const", bufs=1) as cpool,
            tc.tile_pool(name="wts", bufs=1) as wpool,
            tc.tile_pool(name="xp", bufs=1) as xpool,
            tc.tile_pool(name="hb", bufs=1) as hpool,
            tc.tile_pool(name="cm", bufs=1) as cmpool,
            tc.tile_pool(name="wk", bufs=2) as wkpool,
            tc.tile_pool(name="ps", bufs=5, space="PSUM") as pspool,
            tc.tile_pool(name="ps1", bufs=2, space="PSUM") as ps1pool,
        ):
            ones_sb = cpool.tile([128, 128], f32)
            ones_bf = cpool.tile([COUT, 1], bf16)
            nc.sync.dma_start(out=ones_sb[:], in_=ones_ext[:])
            nc.vector.tensor_copy(ones_bf[:], ones_sb[:COUT, 0:1])

            # ---- weights: host-prepped (BN-scale folded, transposed, bf16
            # hi/lo split) -> just DMA in ----
            waT_hi = wpool.tile([128, 18, CMID], bf16)
            waT_lo = wpool.tile([128, 18, CMID], bf16)
            nc.sync.dma_start(out=waT_hi[:], in_=wah_ext[:])
            nc.scalar.dma_start(out=waT_lo[:], in_=wal_ext[:])
            wbT_hi = wpool.tile([128, 2, COUT], bf16)
            wbT_lo = wpool.tile([128, 2, COUT], bf16)
            nc.sync.dma_start(out=wbT_hi[:], in_=wbh_ext[:])
            nc.sync.dma_start(out=wbT_lo[:], in_=wbl_ext[:])
            ba_sb = wpool.tile([128, 2], f32)
            nc.sync.dma_start(
                out=ba_sb[:], in_=ba_ext.ap().rearrange("(m p) -> p m", p=128)
            )
            bb_sb = wpool.tile([COUT, 1], f32)
            nc.sync.dma_start(out=bb_sb[:], in_=bb_ext[:])

            # padded input tiles (border zeroed once; interior rewritten per sample)
            x_ph = xpool.tile([128, 2, PPIX], bf16)
            x_pl = xpool.tile([128, 2, PPIX], bf16)
            nc.vector.memset(x_ph[:], 0.0)
            nc.vector.memset(x_pl[:], 0.0)

            h_hi = hpool.tile([128, 2, PIX], bf16)
            h_lo = hpool.tile([128, 2, PIX], bf16)
            logits_cm = cmpool.tile([COUT, PIX], f32)
            s_row = cmpool.tile([1, PIX], f32)
            eh_cm = cmpool.tile([COUT, PIX], bf16)
            el_cm = cmpool.tile([COUT, PIX], bf16)

            for b in range(B_PER_CORE):
                work_cm = wkpool.tile([COUT, PIX], f32, tag="work")
                # load x hi/lo into padded interiors, spread across DMA queues
                xdma = (nc.gpsimd, nc.sync, nc.scalar, nc.gpsimd)
                for i, (xt, xe) in enumerate(
                    ((x_ph, xh_ext), (x_ph, xh_ext), (x_pl, xl_ext), (x_pl, xl_ext))
                ):
                    k = i % 2
                    xdma[i].dma_start(
                        out=xt[:, k, :].rearrange("p (hh ww) -> p hh ww", hh=HP)[
                            :, 1 : 1 + H, 1 : 1 + W
                        ],
                        in_=xe.ap()[b].rearrange("(k p) hh ww -> k p hh ww", p=128)[
                            k
                        ],
                    )
                # ---- conv-a (3x3) + BN + ReLU ----
                for m in range(2):
                    for tl in range(NTILE):
                        pa = pspool.tile([128, TILE_PIX], f32, tag="mm")
                        y0 = tl * TROWS
                        n_mm = 0
                        # pass-major order: hi*x_hi taps first so the first
                        # matmuls only depend on the x_hi DMAs
                        for wT, xt in (
                            (waT_hi, x_ph),
                            (waT_hi, x_pl),
                            (waT_lo, x_ph),
                        ):
                            for k in range(2):
                                for t in range(9):
                                    dy, dx = t // 3, t % 3
                                    rhs = xt[:, k, :].rearrange(
                                        "p (hh ww) -> p hh ww", hh=HP
                                    )[:, y0 + dy : y0 + dy + TROWS, dx : dx + W]
                                    nc.tensor.matmul(
                                        pa[:],
                                        wT[:, k * 9 + t, m * 128 : (m + 1) * 128],
                                        rhs,
                                        start=(n_mm == 0),
                                        stop=(n_mm == 53),
                                    )
                                    n_mm += 1
                        hs = wkpool.tile([128, TILE_PIX], f32, tag="hscr")
                        hs2 = wkpool.tile([128, TILE_PIX], f32, tag="hscr2")
                        sl = slice(tl * TILE_PIX, (tl + 1) * TILE_PIX)
                        nc.scalar.activation(
                            hs[:],
                            pa[:],
                            mybir.ActivationFunctionType.Relu,
                            bias=ba_sb[:, m : m + 1],
                            scale=1.0,
                        )
                        nc.scalar.copy(h_hi[:, m, sl], hs[:])
                        nc.vector.tensor_copy(hs2[:], h_hi[:, m, sl])
                        nc.vector.tensor_sub(hs2[:], hs[:], hs2[:])
                        nc.vector.tensor_copy(h_lo[:, m, sl], hs2[:])

                # ---- conv-b (1x1) + BN  (channel-major) ----
                for tl in range(NTILE):
                    pb = pspool.tile([COUT, TILE_PIX], f32, tag="mm")
                    n_mm = 0
                    for k in range(2):
                        hsl = slice(tl * TILE_PIX, (tl + 1) * TILE_PIX)
                        for lhs, rhs in (
                            (wbT_hi[:, k, :], h_hi[:, k, hsl]),
                            (wbT_hi[:, k, :], h_lo[:, k, hsl]),
                            (wbT_lo[:, k, :], h_hi[:, k, hsl]),
                        ):
                            nc.tensor.matmul(
                                pb[:], lhs, rhs, start=(n_mm == 0), stop=(n_mm == 5)
                            )
                            n_mm += 1
                    nc.vector.tensor_scalar_add(
                        logits_cm[:, tl * TILE_PIX : (tl + 1) * TILE_PIX],
                        pb[:],
                        bb_sb[:],
                    )
                nc.sync.dma_start(
                    out=logits_out.ap()[b].rearrange("c hh ww -> c (hh ww)"),
                    in_=logits_cm[:],
                )

                # ---- softmax over channel dim ----
                for tl in range(NTILE):
                    psum_s = ps1pool.tile([1, TILE_PIX], f32, tag="sm")
                    sl = slice(tl * TILE_PIX, (tl + 1) * TILE_PIX)
                    nc.scalar.activation(
                        work_cm[:, sl],
                        logits_cm[:, sl],
                        mybir.ActivationFunctionType.Exp,
                    )
                    nc.scalar.copy(eh_cm[:, sl], work_cm[:, sl])
                    nc.vector.tensor_sub(el_cm[:, sl], work_cm[:, sl], eh_cm[:, sl])
                    nc.tensor.matmul(
                        psum_s[:], ones_bf[:], eh_cm[:, sl], start=True, stop=False
                    )
                    nc.tensor.matmul(
                        psum_s[:], ones_bf[:], el_cm[:, sl], start=False, stop=True
                    )
                    nc.vector.tensor_copy(s_row[:, sl], psum_s[:])
                nc.vector.reciprocal(s_row[:], s_row[:])
                # broadcast 1/S to all channel partitions: bounce via DRAM with a
                # 0-stride read on the DRAM side; logits_cm is dead after its
                # output DMA, so reuse it as the broadcast target
                nc.sync.dma_start(out=s_dram.ap()[b][None, :], in_=s_row[:])
                nc.sync.dma_start(
                    out=logits_cm[:],
                    in_=s_dram.ap()[b][None, :].broadcast_to([COUT, PIX]),
                )
                nc.vector.tensor_mul(work_cm[:], work_cm[:], logits_cm[:])

                # ---- heat output: pixel shuffle via DMA access pattern ----
                # heat[b, hc*8+r, wc*8+cc] = prob[r*8+cc, hc*80+wc]
                dma_engines = (nc.sync, nc.scalar, nc.gpsimd)
                for r in range(GRID):
                    for cc in range(GRID):
                        c = r * GRID + cc
                        dma_engines[c % len(dma_engines)].dma_start(
                            out=heat_out.ap()[b].rearrange(
                                "(hc r) (wc cc) -> r cc hc wc", r=GRID, cc=GRID
                            )[r, cc],
                            in_=work_cm[c : c + 1, :].rearrange(
                                "c (hc wc) -> c hc wc", hc=H
                            ),
                        )

    nc.finalize()
    return nc


def _get_nc():
    if "nc" not in _CACHED:
        _CACHED["nc"] = _build_nc()
    return _CACHED["nc"]


def _host_nms(heat):
    """Exact replication of reference _box_nms (vectorized, Jacobi to fixpoint)."""
    B = heat.shape[0]
    flat = heat.reshape(B, -1)
    # top-1024 sorted desc, ties by index asc (matches jax top_k)
    idx = np.argsort(-flat, axis=1, kind="stable")[:, :NMS_CAND]
    scores = np.take_along_axis(flat, idx, axis=1)
    ys = (idx // HW_).astype(np.float32)
    xs = (idx % HW_).astype(np.float32)
    heat_nms = np.zeros_like(flat)
    for b in range(B):
        dy = np.abs(ys[b][:, None] - ys[b][None, :])
        dx = np.abs(xs[b][:, None] - xs[b][None, :])
        inter = np.maximum(NMS_SIZE - dy, 0.0) * np.maximum(NMS_SIZE - dx, 0.0)
        iou = inter / (2.0 * NMS_SIZE * NMS_SIZE - inter)
        overlap = iou > IOU_TH
        valid = scores[b] > MIN_PROB
        np.fill_diagonal(overlap, False)
        ov_ut = np.triu(overlap, 1)
        keep = valid.copy()
        for _ in range(NMS_CAND + 1):  # Jacobi fixpoint == greedy result;
            supp = ov_ut[keep].any(axis=0)  # converges in <= chain depth iters
            newkeep = valid & ~supp
            if (newkeep == keep).all():
                break
            keep = newkeep
        rank = np.cumsum(keep)
        keep = keep & (rank <= TOP_K)
        kept = np.where(keep, scores[b], 0.0).astype(np.float32)
        heat_nms[b, idx[b]] = kept
    return heat_nms.reshape(B, HH, HW_)


def kernel(x, Wa, ba, ga, bta, ma, va, Wb, bb, gb, btb, mb, vb):
    from concourse.bass_utils import run_bass_kernel_spmd

    import ml_dtypes

    x = np.ascontiguousarray(np.asarray(x, dtype=np.float32))
    x_hi = x.astype(ml_dtypes.bfloat16)
    x_lo = (x - x_hi.astype(np.float32)).astype(ml_dtypes.bfloat16)
    nc = _get_nc()

    # fold BN params on host (cheap per-channel math, not data-dependent)
    def bn_fold(g, v, m_, bt, bconv):
        g = np.asarray(g, np.float32)
        v = np.asarray(v, np.float32)
        m_ = np.asarray(m_, np.float32)
        bt = np.asarray(bt, np.float32)
        bconv = np.asarray(bconv, np.float32)
        scale = (g * (1.0 / np.sqrt(v + np.float32(EPS)))).astype(np.float32)
        bias = ((bconv - m_) * scale + bt).astype(np.float32)
        return scale, bias

    scale_a, bias_a = bn_fold(ga, va, ma, bta, ba)
    scale_b, bias_b = bn_fold(gb, vb, mb, btb, bb)

    # fold BN scale, transpose to [ci, (k,t), co] lhsT layout, split bf16 hi/lo
    WaS = np.asarray(Wa, np.float32).reshape(CMID, CIN, 9) * scale_a[:, None, None]
    WaT = WaS.transpose(1, 2, 0).reshape(2, 128, 9, CMID)  # [k, ci_p, t, co]
    WaT = np.ascontiguousarray(WaT.transpose(1, 0, 2, 3).reshape(128, 18, CMID))
    WaT_hi = WaT.astype(ml_dtypes.bfloat16)
    WaT_lo = (WaT - WaT_hi.astype(np.float32)).astype(ml_dtypes.bfloat16)
    WbS = np.asarray(Wb, np.float32).reshape(COUT, CMID) * scale_b[:, None]
    WbT = np.ascontiguousarray(
        WbS.transpose(1, 0).reshape(2, 128, COUT).transpose(1, 0, 2)
    )
    WbT_hi = WbT.astype(ml_dtypes.bfloat16)
    WbT_lo = (WbT - WbT_hi.astype(np.float32)).astype(ml_dtypes.bfloat16)

    ones128 = np.ones((128, 128), np.float32)

    in_maps = []
    for c in range(8):
        in_maps.append(
            {
                "x_hi": x_hi[c * B_PER_CORE : (c + 1) * B_PER_CORE],
                "x_lo": x_lo[c * B_PER_CORE : (c + 1) * B_PER_CORE],
                "waT_hi": WaT_hi,
                "waT_lo": WaT_lo,
                "wbT_hi": WbT_hi,
                "wbT_lo": WbT_lo,
                "bias_a": bias_a,
                "bias_b": bias_b.reshape(COUT, 1),
                "ones128": ones128,
            }
        )

    _CACHED["last_in_maps"] = in_maps
    res = run_bass_kernel_spmd(nc, in_maps, core_ids=list(range(8)))
    logits = np.concatenate([r["logits"] for r in res.results], axis=0)
    heat = np.concatenate([r["heat"] for r in res.results], axis=0)

    heat_nms = _host_nms(heat)
    pred = (heat_nms >= MIN_PROB).astype(np.int32)
    return logits, heat, heat_nms, pred


# revision 28
# speedup vs baseline: 1.2371x; 1.2371x over previous
"""Trainium2 Bass kernel for nn_Detector_head (SuperPoint-style detector head).

Pipeline per sample: 3x3 conv(256->256)+BN+ReLU -> 1x1 conv(256->65)+BN ->
softmax(65) -> drop dustbin -> pixel_shuffle(8) -> greedy box-NMS -> top-300.

Sharding: pure data parallelism, batch 32 -> 8 cores x 4 samples.
"""

import sys

sys.path.insert(0, "/opt/trn_rl_repo")

import numpy as np

B_PER_CORE = 4
CIN = 256
CMID = 256
COUT = 65
H, W = 60, 80
PIX = H * W  # 4800
HP, WP = H + 2, W + 2  # 62, 82
PPIX = HP * WP  # 5084
GRID = 8
HH, HW_ = H * GRID, W * GRID  # 480, 640
HEAT_N = HH * HW_  # 307200
NTILE = 10  # pixel tiles for conv (480 each)
TILE_PIX = PIX // NTILE  # 480
TROWS = TILE_PIX // W  # 6 rows per tile
EPS = 1e-5

NMS_SIZE = 4.0
IOU_TH = 0.1
MIN_PROB = 0.015
TOP_K = 300
NMS_CAND = 1024

_CACHED = {}


def _last_in_maps_get():
    return _CACHED.get("last_in_maps")


def _build_nc():
    import concourse.bacc as bacc
    import concourse.mybir as mybir
    from concourse.tile import TileContext

    f32 = mybir.dt.float32
    bf16 = mybir.dt.bfloat16
    nc = bacc.Bacc("TRN2", target_bir_lowering=False, debug=False, num_devices=8)

    xh_ext = nc.declare_dram_parameter(
        "x_hi", [B_PER_CORE, CIN, H, W], bf16, isOutput=False
    )
    xl_ext = nc.declare_dram_parameter(
        "x_lo", [B_PER_CORE, CIN, H, W], bf16, isOutput=False
    )
    wah_ext = nc.declare_dram_parameter(
        "waT_hi", [128, 18, CMID], bf16, isOutput=False
    )
    wal_ext = nc.declare_dram_parameter(
        "waT_lo", [128, 18, CMID], bf16, isOutput=False
    )
    wbh_ext = nc.declare_dram_parameter("wbT_hi", [128, 2, COUT], bf16, isOutput=False)
    wbl_ext = nc.declare_dram_parameter("wbT_lo", [128, 2, COUT], bf16, isOutput=False)
    ba_ext = nc.declare_dram_parameter("bias_a", [CMID], f32, isOutput=False)
    bb_ext = nc.declare_dram_parameter("bias_b", [COUT, 1], f32, isOutput=False)
    ones_ext = nc.declare_dram_parameter("ones128", [128, 128], f32, isOutput=False)

    s_dram = nc.dram_tensor("s_scratch", [B_PER_CORE, PIX], f32)
    logits_out = nc.declare_dram_parameter(
        "logits", [B_PER_CORE, COUT, H, W], f32, isOutput=True
    )
    heat_out = nc.declare_dram_parameter(
        "heat", [B_PER_CORE, HH, HW_], f32, isOutput=True
    )

    with TileContext(nc) as tc:
        with (
            tc.tile_pool(name="const", bufs=1) as cpool,
            tc.tile_pool(name="wts", bufs=1) as wpool,
            tc.tile_pool(name="xp", bufs=1) as xpool,
            tc.tile_pool(name="hb", bufs=1) as hpool,
            tc.tile_pool(name="cm", bufs=1) as cmpool,
            tc.tile_pool(name="wk", bufs=2) as wkpool,
            tc.tile_pool(name="ps", bufs=5, space="PSUM") as pspool,
            tc.tile_pool(name="ps1", bufs=2, space="PSUM") as ps1pool,
        ):
            ones_sb = cpool.tile([128, 128], f32)
            ones_bf = cpool.tile([COUT, 1], bf16)
            nc.sync.dma_start(out=ones_sb[:], in_=ones_ext[:])
            nc.vector.tensor_copy(ones_bf[:], ones_sb[:COUT, 0:1])

            # ---- weights: host-prepped (BN-scale folded, transposed, bf16
            # hi/lo split) -> just DMA in ----
            waT_hi = wpool.tile([128, 18, CMID], bf16)
            waT_lo = wpool.tile([128, 18, CMID], bf16)
            nc.sync.dma_start(out=waT_hi[:], in_=wah_ext[:])
            nc.scalar.dma_start(out=waT_lo[:], in_=wal_ext[:])
            wbT_hi = wpool.tile([128, 2, COUT], bf16)
            wbT_lo = wpool.tile([128, 2, COUT], bf16)
            nc.sync.dma_start(out=wbT_hi[:], in_=wbh_ext[:])
            nc.sync.dma_start(out=wbT_lo[:], in_=wbl_ext[:])
            ba_sb = wpool.tile([128, 2], f32)
            nc.sync.dma_start(
                out=ba_sb[:], in_=ba_ext.ap().rearrange("(m p) -> p m", p=128)
            )
            bb_sb = wpool.tile([COUT, 1], f32)
            nc.sync.dma_start(out=bb_sb[:], in_=bb_ext[:])

            # padded input tiles (border zeroed once; interior rewritten per sample)
            x_ph = xpool.tile([128, 2, PPIX], bf16)
            x_pl = xpool.tile([128, 2, PPIX], bf16)
            nc.vector.memset(x_ph[:], 0.0)
            nc.vector.memset(x_pl[:], 0.0)

            h_hi = hpool.tile([128, 2, PIX], bf16)
            h_lo = hpool.tile([128, 2, PIX], bf16)
            logits_cm = cmpool.tile([COUT, PIX], f32)
            s_row = cmpool.tile([1, PIX], f32)
            eh_cm = cmpool.tile([COUT, PIX], bf16)
            el_cm = cmpool.tile([COUT, PIX], bf16)

            for b in range(B_PER_CORE):
                work_cm = wkpool.tile([COUT, PIX], f32, tag="work")
                # load x hi/lo into padded interiors; split by row-half across
                # the three DMA queues so the first conv taps (k=0) start early
                xdma = (nc.gpsimd, nc.sync, nc.scalar)
                i = 0
                for xt, xe in ((x_ph, xh_ext), (x_pl, xl_ext)):
                    for k in range(2):
                        for r0 in (0, H // 2):
                            xdma[i % 3].dma_start(
                                out=xt[:, k, :].rearrange(
                                    "p (hh ww) -> p hh ww", hh=HP
                                )[:, 1 + r0 : 1 + r0 + H // 2, 1 : 1 + W],
                                in_=xe.ap()[b].rearrange(
                                    "(k p) hh ww -> k p hh ww", p=128
                                )[k, :, r0 : r0 + H // 2],
                            )
                            i += 1
                # ---- conv-a (3x3) + BN + ReLU ----
                for m in range(2):
                    for tl in range(NTILE):
                        pa = pspool.tile([128, TILE_PIX], f32, tag="mm")
                        y0 = tl * TROWS
                        n_mm = 0
                        # pass-major order: hi*x_hi taps first so the first
                        # matmuls only depend on the x_hi DMAs
                        for wT, xt in (
                            (waT_hi, x_ph),
                            (waT_hi, x_pl),
                            (waT_lo, x_ph),
                        ):
                            for k in range(2):
                                for t in range(9):
                                    dy, dx = t // 3, t % 3
                                    rhs = xt[:, k, :].rearrange(
                                        "p (hh ww) -> p hh ww", hh=HP
                                    )[:, y0 + dy : y0 + dy + TROWS, dx : dx + W]
                                    nc.tensor.matmul(
                                        pa[:],
                                        wT[:, k * 9 + t, m * 128 : (m + 1) * 128],
                                        rhs,
                                        start=(n_mm == 0),
                                        stop=(n_mm == 53),
                                    )
                                    n_mm += 1
                        hs = wkpool.tile([128, TILE_PIX], f32, tag="hscr")
                        hs2 = wkpool.tile([128, TILE_PIX], f32, tag="hscr2")
                        sl = slice(tl * TILE_PIX, (tl + 1) * TILE_PIX)
                        nc.scalar.activation(
                            hs[:],
                            pa[:],
                            mybir.ActivationFunctionType.Relu,
                            bias=ba_sb[:, m : m + 1],
                            scale=1.0,
                        )
                        nc.scalar.copy(h_hi[:, m, sl], hs[:])
                        nc.vector.tensor_copy(hs2[:], h_hi[:, m, sl])
                        nc.vector.tensor_sub(hs2[:], hs[:], hs2[:])
                        nc.vector.tensor_copy(h_lo[:, m, sl], hs2[:])

                # ---- conv-b (1x1) + BN  (channel-major) ----
                for tl in range(NTILE):
                    pb = pspool.tile([COUT, TILE_PIX], f32, tag="mm")
                    n_mm = 0
                    for k in range(2):
                        hsl = slice(tl * TILE_PIX, (tl + 1) * TILE_PIX)
                        for lhs, rhs in (
                            (wbT_hi[:, k, :], h_hi[:, k, hsl]),
                            (wbT_hi[:, k, :], h_lo[:, k, hsl]),
                            (wbT_lo[:, k, :], h_hi[:, k, hsl]),
                        ):
                            nc.tensor.matmul(
                                pb[:], lhs, rhs, start=(n_mm == 0), stop=(n_mm == 5)
                            )
                            n_mm += 1
                    nc.vector.tensor_scalar_add(
                        logits_cm[:, tl * TILE_PIX : (tl + 1) * TILE_PIX],
                        pb[:],
                        bb_sb[:],
                    )
                nc.sync.dma_start(
                    out=logits_out.ap()[b].rearrange("c hh ww -> c (hh ww)"),
                    in_=logits_cm[:],
                )

                # ---- softmax over channel dim ----
                for tl in range(NTILE):
                    psum_s = ps1pool.tile([1, TILE_PIX], f32, tag="sm")
                    sl = slice(tl * TILE_PIX, (tl + 1) * TILE_PIX)
                    nc.scalar.activation(
                        work_cm[:, sl],
                        logits_cm[:, sl],
                        mybir.ActivationFunctionType.Exp,
                    )
                    nc.scalar.copy(eh_cm[:, sl], work_cm[:, sl])
                    nc.vector.tensor_sub(el_cm[:, sl], work_cm[:, sl], eh_cm[:, sl])
                    nc.tensor.matmul(
                        psum_s[:], ones_bf[:], eh_cm[:, sl], start=True, stop=False
                    )
                    nc.tensor.matmul(
                        psum_s[:], ones_bf[:], el_cm[:, sl], start=False, stop=True
                    )
                    nc.vector.tensor_copy(s_row[:, sl], psum_s[:])
                nc.vector.reciprocal(s_row[:], s_row[:])
                # broadcast 1/S to all channel partitions: bounce via DRAM with a
                # 0-stride read on the DRAM side; logits_cm is dead after its
                # output DMA, so reuse it as the broadcast target
                nc.sync.dma_start(out=s_dram.ap()[b][None, :], in_=s_row[:])
                nc.sync.dma_start(
                    out=logits_cm[:],
                    in_=s_dram.ap()[b][None, :].broadcast_to([COUT, PIX]),
                )
                nc.vector.tensor_mul(work_cm[:], work_cm[:], logits_cm[:])

                # ---- heat output: pixel shuffle via DMA access pattern ----
                # heat[b, hc*8+r, wc*8+cc] = prob[r*8+cc, hc*80+wc]
                dma_engines = (nc.sync, nc.scalar, nc.gpsimd)
                for r in range(GRID):
                    for cc in range(GRID):
                        c = r * GRID + cc
                        dma_engines[c % len(dma_engines)].dma_start(
                            out=heat_out.ap()[b].rearrange(
                                "(hc r) (wc cc) -> r cc hc wc", r=GRID, cc=GRID
                            )[r, cc],
                            in_=work_cm[c : c + 1, :].rearrange(
                                "c (hc wc) -> c hc wc", hc=H
                            ),
                        )

    nc.finalize()
    return nc


def _get_nc():
    if "nc" not in _CACHED:
        _CACHED["nc"] = _build_nc()
    return _CACHED["nc"]


def _host_nms(heat):
    """Exact replication of reference _box_nms (vectorized, Jacobi to fixpoint)."""
    B = heat.shape[0]
    flat = heat.reshape(B, -1)
    # top-1024 sorted desc, ties by index asc (matches jax top_k)
    idx = np.argsort(-flat, axis=1, kind="stable")[:, :NMS_CAND]
    scores = np.take_along_axis(flat, idx, axis=1)
    ys = (idx // HW_).astype(np.float32)
    xs = (idx % HW_).astype(np.float32)
    heat_nms = np.zeros_like(flat)
    for b in range(B):
        dy = np.abs(ys[b][:, None] - ys[b][None, :])
        dx = np.abs(xs[b][:, None] - xs[b][None, :])
        inter = np.maximum(NMS_SIZE - dy, 0.0) * np.maximum(NMS_SIZE - dx, 0.0)
        iou = inter / (2.0 * NMS_SIZE * NMS_SIZE - inter)
        overlap = iou > IOU_TH
        valid = scores[b] > MIN_PROB
        np.fill_diagonal(overlap, False)
        ov_ut = np.triu(overlap, 1)
        keep = valid.copy()
        for _ in range(NMS_CAND + 1):  # Jacobi fixpoint == greedy result;
            supp = ov_ut[keep].any(axis=0)  # converges in <= chain depth iters
            newkeep = valid & ~supp
            if (newkeep == keep).all():
                break
            keep = newkeep
        rank = np.cumsum(keep)
        keep = keep & (rank <= TOP_K)
        kept = np.where(keep, scores[b], 0.0).astype(np.float32)
        heat_nms[b, idx[b]] = kept
    return heat_nms.reshape(B, HH, HW_)


def kernel(x, Wa, ba, ga, bta, ma, va, Wb, bb, gb, btb, mb, vb):
    from concourse.bass_utils import run_bass_kernel_spmd

    import ml_dtypes

    x = np.ascontiguousarray(np.asarray(x, dtype=np.float32))
    x_hi = x.astype(ml_dtypes.bfloat16)
    x_lo = (x - x_hi.astype(np.float32)).astype(ml_dtypes.bfloat16)
    nc = _get_nc()

    # fold BN params on host (cheap per-channel math, not data-dependent)
    def bn_fold(g, v, m_, bt, bconv):
        g = np.asarray(g, np.float32)
        v = np.asarray(v, np.float32)
        m_ = np.asarray(m_, np.float32)
        bt = np.asarray(bt, np.float32)
        bconv = np.asarray(bconv, np.float32)
        scale = (g * (1.0 / np.sqrt(v + np.float32(EPS)))).astype(np.float32)
        bias = ((bconv - m_) * scale + bt).astype(np.float32)
        return scale, bias

    scale_a, bias_a = bn_fold(ga, va, ma, bta, ba)
    scale_b, bias_b = bn_fold(gb, vb, mb, btb, bb)

    # fold BN scale, transpose to [ci, (k,t), co] lhsT layout, split bf16 hi/lo
    WaS = np.asarray(Wa, np.float32).reshape(CMID, CIN, 9) * scale_a[:, None, None]
    WaT = WaS.transpose(1, 2, 0).reshape(2, 128, 9, CMID)  # [k, ci_p, t, co]
    WaT = np.ascontiguousarray(WaT.transpose(1, 0, 2, 3).reshape(128, 18, CMID))
    WaT_hi = WaT.astype(ml_dtypes.bfloat16)
    WaT_lo = (WaT - WaT_hi.astype(np.float32)).astype(ml_dtypes.bfloat16)
    WbS = np.asarray(Wb, np.float32).reshape(COUT, CMID) * scale_b[:, None]
    WbT = np.ascontiguousarray(
        WbS.transpose(1, 0).reshape(2, 128, COUT).transpose(1, 0, 2)
    )
    WbT_hi = WbT.astype(ml_dtypes.bfloat16)
    WbT_lo = (WbT - WbT_hi.astype(np.float32)).astype(ml_dtypes.bfloat16)

    ones128 = np.ones((128, 128), np.float32)

    in_maps = []
    for c in range(8):
        in_maps.append(
            {
                "x_hi": x_hi[c * B_PER_CORE : (c + 1) * B_PER_CORE],
                "x_lo": x_lo[c * B_PER_CORE : (c + 1) * B_PER_CORE],
                "waT_hi": WaT_hi,
                "waT_lo": WaT_lo,
                "wbT_hi": WbT_hi,
                "wbT_lo": WbT_lo,
                "bias_a": bias_a,
                "bias_b": bias_b.reshape(COUT, 1),
                "ones128": ones128,
            }
        )

    _CACHED["last_in_maps"] = in_maps
    res = run_bass_kernel_spmd(nc, in_maps, core_ids=list(range(8)))
    logits = np.concatenate([r["logits"] for r in res.results], axis=0)
    heat = np.concatenate([r["heat"] for r in res.results], axis=0)

    heat_nms = _host_nms(heat)
    pred = (heat_nms >= MIN_PROB).astype(np.int32)
    return logits, heat, heat_nms, pred


# revision 30
# speedup vs baseline: 1.3329x; 1.0774x over previous
"""Trainium2 Bass kernel for nn_Detector_head (SuperPoint-style detector head).

Pipeline per sample: 3x3 conv(256->256)+BN+ReLU -> 1x1 conv(256->65)+BN ->
softmax(65) -> drop dustbin -> pixel_shuffle(8) -> greedy box-NMS -> top-300.

Sharding: pure data parallelism, batch 32 -> 8 cores x 4 samples.
"""

import sys

sys.path.insert(0, "/opt/trn_rl_repo")

import numpy as np

B_PER_CORE = 4
CIN = 256
CMID = 256
COUT = 65
H, W = 60, 80
PIX = H * W  # 4800
HP, WP = H + 2, W + 2  # 62, 82
PPIX = HP * WP  # 5084
GRID = 8
HH, HW_ = H * GRID, W * GRID  # 480, 640
HEAT_N = HH * HW_  # 307200
NTILE = 10  # pixel tiles for conv (480 each)
TILE_PIX = PIX // NTILE  # 480
TROWS = TILE_PIX // W  # 6 rows per tile
EPS = 1e-5

NMS_SIZE = 4.0
IOU_TH = 0.1
MIN_PROB = 0.015
TOP_K = 300
NMS_CAND = 1024

_CACHED = {}


def _last_in_maps_get():
    return _CACHED.get("last_in_maps")


def _build_nc():
    import concourse.bacc as bacc
    import concourse.mybir as mybir
    from concourse.tile import TileContext

    f32 = mybir.dt.float32
    bf16 = mybir.dt.bfloat16
    nc = bacc.Bacc("TRN2", target_bir_lowering=False, debug=False, num_devices=8)

    xh_ext = nc.declare_dram_parameter(
        "x_hi", [B_PER_CORE, CIN, H, W], bf16, isOutput=False
    )
    xl_ext = nc.declare_dram_parameter(
        "x_lo", [B_PER_CORE, CIN, H, W], bf16, isOutput=False
    )
    wah_ext = nc.declare_dram_parameter(
        "waT_hi", [128, 18, CMID], bf16, isOutput=False
    )
    wal_ext = nc.declare_dram_parameter(
        "waT_lo", [128, 18, CMID], bf16, isOutput=False
    )
    wbh_ext = nc.declare_dram_parameter("wbT_hi", [128, 2, COUT], bf16, isOutput=False)
    wbl_ext = nc.declare_dram_parameter("wbT_lo", [128, 2, COUT], bf16, isOutput=False)
    ba_ext = nc.declare_dram_parameter("bias_a", [CMID], f32, isOutput=False)
    bb_ext = nc.declare_dram_parameter("bias_b", [COUT, 1], f32, isOutput=False)
    ones_ext = nc.declare_dram_parameter("ones128", [128, 128], f32, isOutput=False)

    s_dram = nc.dram_tensor("s_scratch", [B_PER_CORE, PIX], f32)
    logits_out = nc.declare_dram_parameter(
        "logits", [B_PER_CORE, COUT, H, W], f32, isOutput=True
    )
    heat_out = nc.declare_dram_parameter(
        "heat", [B_PER_CORE, HH, HW_], f32, isOutput=True
    )

    with TileContext(nc) as tc:
        with (
            tc.tile_pool(name="const", bufs=1) as cpool,
            tc.tile_pool(name="wts", bufs=1) as wpool,
            tc.tile_pool(name="xp", bufs=1) as xpool,
            tc.tile_pool(name="hb", bufs=1) as hpool,
            tc.tile_pool(name="cm", bufs=1) as cmpool,
            tc.tile_pool(name="wk", bufs=2) as wkpool,
            tc.tile_pool(name="ps", bufs=5, space="PSUM") as pspool,
            tc.tile_pool(name="ps1", bufs=2, space="PSUM") as ps1pool,
        ):
            ones_sb = cpool.tile([128, 128], f32)
            ones_bf = cpool.tile([COUT, 1], bf16)
            nc.sync.dma_start(out=ones_sb[:], in_=ones_ext[:])
            nc.vector.tensor_copy(ones_bf[:], ones_sb[:COUT, 0:1])

            # ---- weights: host-prepped (BN-scale folded, transposed, bf16
            # hi/lo split) -> just DMA in ----
            waT_hi = wpool.tile([128, 18, CMID], bf16)
            waT_lo = wpool.tile([128, 18, CMID], bf16)
            nc.sync.dma_start(out=waT_hi[:], in_=wah_ext[:])
            nc.scalar.dma_start(out=waT_lo[:], in_=wal_ext[:])
            wbT_hi = wpool.tile([128, 2, COUT], bf16)
            wbT_lo = wpool.tile([128, 2, COUT], bf16)
            nc.sync.dma_start(out=wbT_hi[:], in_=wbh_ext[:])
            nc.sync.dma_start(out=wbT_lo[:], in_=wbl_ext[:])
            ba_sb = wpool.tile([128, 2], f32)
            nc.sync.dma_start(
                out=ba_sb[:], in_=ba_ext.ap().rearrange("(m p) -> p m", p=128)
            )
            bb_sb = wpool.tile([COUT, 1], f32)
            nc.sync.dma_start(out=bb_sb[:], in_=bb_ext[:])

            # padded input tiles (border zeroed once; interior rewritten per sample)
            x_ph = xpool.tile([128, 2, PPIX], bf16)
            x_pl = xpool.tile([128, 2, PPIX], bf16)
            nc.vector.memset(x_ph[:], 0.0)
            nc.vector.memset(x_pl[:], 0.0)

            h_hi = hpool.tile([128, 2, PIX], bf16)
            h_lo = hpool.tile([128, 2, PIX], bf16)
            logits_cm = cmpool.tile([COUT, PIX], f32)
            s_row = cmpool.tile([1, PIX], f32)
            eh_cm = cmpool.tile([COUT, PIX], bf16)
            el_cm = cmpool.tile([COUT, PIX], bf16)

            for b in range(B_PER_CORE):
                work_cm = wkpool.tile([COUT, PIX], f32, tag="work")
                # load x hi/lo into padded interiors; split by row-half across
                # the three DMA queues so the first conv taps (k=0) start early
                xdma = (nc.gpsimd, nc.sync, nc.scalar)
                i = 0
                for xt, xe in ((x_ph, xh_ext), (x_pl, xl_ext)):
                    for k in range(2):
                        for r0 in (0, H // 2):
                            xdma[i % 3].dma_start(
                                out=xt[:, k, :].rearrange(
                                    "p (hh ww) -> p hh ww", hh=HP
                                )[:, 1 + r0 : 1 + r0 + H // 2, 1 : 1 + W],
                                in_=xe.ap()[b].rearrange(
                                    "(k p) hh ww -> k p hh ww", p=128
                                )[k, :, r0 : r0 + H // 2],
                            )
                            i += 1
                # ---- conv-a (3x3) + BN + ReLU ----
                for m in range(2):
                    for tl in range(NTILE):
                        pa = pspool.tile([128, TILE_PIX], f32, tag="mm")
                        y0 = tl * TROWS
                        n_mm = 0
                        # pass-major order: hi*x_hi taps first so the first
                        # matmuls only depend on the x_hi DMAs
                        for wT, xt in (
                            (waT_hi, x_ph),
                            (waT_hi, x_pl),
                            (waT_lo, x_ph),
                        ):
                            for k in range(2):
                                for t in range(9):
                                    dy, dx = t // 3, t % 3
                                    rhs = xt[:, k, :].rearrange(
                                        "p (hh ww) -> p hh ww", hh=HP
                                    )[:, y0 + dy : y0 + dy + TROWS, dx : dx + W]
                                    nc.tensor.matmul(
                                        pa[:],
                                        wT[:, k * 9 + t, m * 128 : (m + 1) * 128],
                                        rhs,
                                        start=(n_mm == 0),
                                        stop=(n_mm == 53),
                                    )
                                    n_mm += 1
                        hs = wkpool.tile([128, TILE_PIX], f32, tag="hscr")
                        hs2 = wkpool.tile([128, TILE_PIX], f32, tag="hscr2")
                        sl = slice(tl * TILE_PIX, (tl + 1) * TILE_PIX)
                        nc.scalar.activation(
                            hs[:],
                            pa[:],
                            mybir.ActivationFunctionType.Relu,
                            bias=ba_sb[:, m : m + 1],
                            scale=1.0,
                        )
                        nc.scalar.copy(h_hi[:, m, sl], hs[:])
                        nc.vector.tensor_copy(hs2[:], h_hi[:, m, sl])
                        nc.vector.tensor_sub(hs2[:], hs[:], hs2[:])
                        nc.vector.tensor_copy(h_lo[:, m, sl], hs2[:])

                # ---- conv-b (1x1) + BN  (channel-major) ----
                for tl in range(NTILE):
                    pb = pspool.tile([COUT, TILE_PIX], f32, tag="mm")
                    n_mm = 0
                    for k in range(2):
                        hsl = slice(tl * TILE_PIX, (tl + 1) * TILE_PIX)
                        for lhs, rhs in (
                            (wbT_hi[:, k, :], h_hi[:, k, hsl]),
                            (wbT_hi[:, k, :], h_lo[:, k, hsl]),
                            (wbT_lo[:, k, :], h_hi[:, k, hsl]),
                        ):
                            nc.tensor.matmul(
                                pb[:], lhs, rhs, start=(n_mm == 0), stop=(n_mm == 5)
                            )
                            n_mm += 1
                    nc.vector.tensor_scalar_add(
                        logits_cm[:, tl * TILE_PIX : (tl + 1) * TILE_PIX],
                        pb[:],
                        bb_sb[:],
                    )
                nc.sync.dma_start(
                    out=logits_out.ap()[b].rearrange("c hh ww -> c (hh ww)"),
                    in_=logits_cm[:],
                )

                # ---- softmax over channel dim ----
                for tl in range(NTILE):
                    psum_s = ps1pool.tile([1, TILE_PIX], f32, tag="sm")
                    sl = slice(tl * TILE_PIX, (tl + 1) * TILE_PIX)
                    nc.scalar.activation(
                        work_cm[:, sl],
                        logits_cm[:, sl],
                        mybir.ActivationFunctionType.Exp,
                    )
                    nc.scalar.copy(eh_cm[:, sl], work_cm[:, sl])
                    nc.vector.tensor_sub(el_cm[:, sl], work_cm[:, sl], eh_cm[:, sl])
                    nc.tensor.matmul(
                        psum_s[:], ones_bf[:], eh_cm[:, sl], start=True, stop=False
                    )
                    nc.tensor.matmul(
                        psum_s[:], ones_bf[:], el_cm[:, sl], start=False, stop=True
                    )
                    nc.vector.tensor_copy(s_row[:, sl], psum_s[:])
                nc.vector.reciprocal(s_row[:], s_row[:])
                # broadcast 1/S to all channel partitions: bounce via DRAM with a
                # 0-stride read on the DRAM side; logits_cm is dead after its
                # output DMA, so reuse it as the broadcast target
                nc.sync.dma_start(out=s_dram.ap()[b][None, :], in_=s_row[:])
                nc.sync.dma_start(
                    out=logits_cm[:],
                    in_=s_dram.ap()[b][None, :].broadcast_to([COUT, PIX]),
                )
                nc.vector.tensor_mul(work_cm[:], work_cm[:], logits_cm[:])

                # ---- heat output: pixel shuffle via DMA access pattern ----
                # heat[b, hc*8+r, wc*8+cc] = prob[r*8+cc, hc*80+wc]
                dma_engines = (nc.sync, nc.scalar, nc.gpsimd)
                for r in range(GRID):
                    for cc in range(GRID):
                        c = r * GRID + cc
                        dma_engines[c % len(dma_engines)].dma_start(
                            out=heat_out.ap()[b].rearrange(
                                "(hc r) (wc cc) -> r cc hc wc", r=GRID, cc=GRID
                            )[r, cc],
                            in_=work_cm[c : c + 1, :].rearrange(
                                "c (hc wc) -> c hc wc", hc=H
                            ),
                        )

    nc.finalize()
    return nc


def _get_nc():
    if "nc" not in _CACHED:
        _CACHED["nc"] = _build_nc()
    return _CACHED["nc"]


def _host_nms(heat):
    """Exact replication of reference _box_nms (vectorized, Jacobi to fixpoint)."""
    B = heat.shape[0]
    flat = heat.reshape(B, -1)
    # top-1024 sorted desc, ties by index asc (matches jax top_k)
    idx = np.argsort(-flat, axis=1, kind="stable")[:, :NMS_CAND]
    scores = np.take_along_axis(flat, idx, axis=1)
    ys = (idx // HW_).astype(np.float32)
    xs = (idx % HW_).astype(np.float32)
    heat_nms = np.zeros_like(flat)
    for b in range(B):
        dy = np.abs(ys[b][:, None] - ys[b][None, :])
        dx = np.abs(xs[b][:, None] - xs[b][None, :])
        inter = np.maximum(NMS_SIZE - dy, 0.0) * np.maximum(NMS_SIZE - dx, 0.0)
        iou = inter / (2.0 * NMS_SIZE * NMS_SIZE - inter)
        overlap = iou > IOU_TH
        valid = scores[b] > MIN_PROB
        np.fill_diagonal(overlap, False)
        ov_ut = np.triu(overlap, 1)
        keep = valid.copy()
        for _ in range(NMS_CAND + 1):  # Jacobi fixpoint == greedy result;
            supp = ov_ut[keep].any(axis=0)  # converges in <= chain depth iters
            newkeep = valid & ~supp
            if (newkeep == keep).all():
                break
            keep = newkeep
        rank = np.cumsum(keep)
        keep = keep & (rank <= TOP_K)
        kept = np.where(keep, scores[b], 0.0).astype(np.float32)
        heat_nms[b, idx[b]] = kept
    return heat_nms.reshape(B, HH, HW_)


def kernel(x, Wa, ba, ga, bta, ma, va, Wb, bb, gb, btb, mb, vb):
    from concourse.bass_utils import run_bass_kernel_spmd

    import ml_dtypes

    x = np.ascontiguousarray(np.asarray(x, dtype=np.float32))
    x_hi = x.astype(ml_dtypes.bfloat16)
    x_lo = (x - x_hi.astype(np.float32)).astype(ml_dtypes.bfloat16)
    nc = _get_nc()

    # fold BN params on host (cheap per-channel math, not data-dependent)
    def bn_fold(g, v, m_, bt, bconv):
        g = np.asarray(g, np.float32)
        v = np.asarray(v, np.float32)
        m_ = np.asarray(m_, np.float32)
        bt = np.asarray(bt, np.float32)
        bconv = np.asarray(bconv, np.float32)
        scale = (g * (1.0 / np.sqrt(v + np.float32(EPS)))).astype(np.float32)
        bias = ((bconv - m_) * scale + bt).astype(np.float32)
        return scale, bias

    scale_a, bias_a = bn_fold(ga, va, ma, bta, ba)
    scale_b, bias_b = bn_fold(gb, vb, mb, btb, bb)

    # fold BN scale, transpose to [ci, (k,t), co] lhsT layout, split bf16 hi/lo
    WaS = np.asarray(Wa, np.float32).reshape(CMID, CIN, 9) * scale_a[:, None, None]
    WaT = WaS.transpose(1, 2, 0).reshape(2, 128, 9, CMID)  # [k, ci_p, t, co]
    WaT = np.ascontiguousarray(WaT.transpose(1, 0, 2, 3).reshape(128, 18, CMID))
    WaT_hi = WaT.astype(ml_dtypes.bfloat16)
    WaT_lo = (WaT - WaT_hi.astype(np.float32)).astype(ml_dtypes.bfloat16)
    WbS = np.asarray(Wb, np.float32).reshape(COUT, CMID) * scale_b[:, None]
    WbT = np.ascontiguousarray(
        WbS.transpose(1, 0).reshape(2, 128, COUT).transpose(1, 0, 2)
    )
    WbT_hi = WbT.astype(ml_dtypes.bfloat16)
    WbT_lo = (WbT - WbT_hi.astype(np.float32)).astype(ml_dtypes.bfloat16)

    ones128 = np.ones((128, 128), np.float32)

    in_maps = []
    for c in range(8):
        in_maps.append(
            {
                "x_hi": x_hi[c * B_PER_CORE : (c + 1) * B_PER_CORE],
                "x_lo": x_lo[c * B_PER_CORE : (c + 1) * B_PER_CORE],
                "waT_hi": WaT_hi,
                "waT_lo": WaT_lo,
                "wbT_hi": WbT_hi,
                "wbT_lo": WbT_lo,
                "bias_a": bias_a,
                "bias_b": bias_b.reshape(COUT, 1),
                "ones128": ones128,
            }
        )

    _CACHED["last_in_maps"] = in_maps
    res = run_bass_kernel_spmd(nc, in_maps, core_ids=list(range(8)))
    logits = np.concatenate([r["logits"] for r in res.results], axis=0)
    heat = np.concatenate([r["heat"] for r in res.results], axis=0)

    heat_nms = _host_nms(heat)
    pred = (heat_nms >= MIN_PROB).astype(np.int32)
    return logits, heat, heat_nms, pred


# revision 31
# speedup vs baseline: 1.3842x; 1.0385x over previous
"""Trainium2 Bass kernel for nn_Detector_head (SuperPoint-style detector head).

Pipeline per sample: 3x3 conv(256->256)+BN+ReLU -> 1x1 conv(256->65)+BN ->
softmax(65) -> drop dustbin -> pixel_shuffle(8) -> greedy box-NMS -> top-300.

Sharding: pure data parallelism, batch 32 -> 8 cores x 4 samples.
"""

import sys

sys.path.insert(0, "/opt/trn_rl_repo")

import numpy as np

B_PER_CORE = 4
CIN = 256
CMID = 256
COUT = 65
H, W = 60, 80
PIX = H * W  # 4800
HP, WP = H + 2, W + 2  # 62, 82
PPIX = HP * WP  # 5084
GRID = 8
HH, HW_ = H * GRID, W * GRID  # 480, 640
HEAT_N = HH * HW_  # 307200
NTILE = 10  # pixel tiles for conv (480 each)
TILE_PIX = PIX // NTILE  # 480
TROWS = TILE_PIX // W  # 6 rows per tile
EPS = 1e-5

NMS_SIZE = 4.0
IOU_TH = 0.1
MIN_PROB = 0.015
TOP_K = 300
NMS_CAND = 1024

_CACHED = {}


def _last_in_maps_get():
    return _CACHED.get("last_in_maps")


def _build_nc():
    import concourse.bacc as bacc
    import concourse.mybir as mybir
    from concourse.tile import TileContext

    f32 = mybir.dt.float32
    bf16 = mybir.dt.bfloat16
    nc = bacc.Bacc("TRN2", target_bir_lowering=False, debug=False, num_devices=8)

    xh_ext = nc.declare_dram_parameter(
        "x_hi", [B_PER_CORE, CIN, H, W], bf16, isOutput=False
    )
    xl_ext = nc.declare_dram_parameter(
        "x_lo", [B_PER_CORE, CIN, H, W], bf16, isOutput=False
    )
    wah_ext = nc.declare_dram_parameter(
        "waT_hi", [128, 18, CMID], bf16, isOutput=False
    )
    wal_ext = nc.declare_dram_parameter(
        "waT_lo", [128, 18, CMID], bf16, isOutput=False
    )
    wbh_ext = nc.declare_dram_parameter("wbT_hi", [128, 2, COUT], bf16, isOutput=False)
    wbl_ext = nc.declare_dram_parameter("wbT_lo", [128, 2, COUT], bf16, isOutput=False)
    ba_ext = nc.declare_dram_parameter("bias_a", [CMID], f32, isOutput=False)
    bb_ext = nc.declare_dram_parameter("bias_b", [COUT, 1], f32, isOutput=False)
    ones_ext = nc.declare_dram_parameter("ones128", [128, 128], f32, isOutput=False)

    s_dram = nc.dram_tensor("s_scratch", [B_PER_CORE, PIX], f32)
    logits_out = nc.declare_dram_parameter(
        "logits", [B_PER_CORE, COUT, H, W], f32, isOutput=True
    )
    heat_out = nc.declare_dram_parameter(
        "heat", [B_PER_CORE, HH, HW_], f32, isOutput=True
    )

    with TileContext(nc) as tc:
        with (
            tc.tile_pool(name="const", bufs=1) as cpool,
            tc.tile_pool(name="wts", bufs=1) as wpool,
            tc.tile_pool(name="xp", bufs=1) as xpool,
            tc.tile_pool(name="hb", bufs=1) as hpool,
            tc.tile_pool(name="cm", bufs=1) as cmpool,
            tc.tile_pool(name="wk", bufs=2) as wkpool,
            tc.tile_pool(name="ps", bufs=5, space="PSUM") as pspool,
            tc.tile_pool(name="ps1", bufs=3, space="PSUM") as ps1pool,
        ):
            ones_sb = cpool.tile([128, 128], f32)
            ones_bf = cpool.tile([COUT, 1], bf16)
            nc.sync.dma_start(out=ones_sb[:], in_=ones_ext[:])
            nc.vector.tensor_copy(ones_bf[:], ones_sb[:COUT, 0:1])

            # ---- weights: host-prepped (BN-scale folded, transposed, bf16
            # hi/lo split) -> just DMA in ----
            waT_hi = wpool.tile([128, 18, CMID], bf16)
            waT_lo = wpool.tile([128, 18, CMID], bf16)
            nc.sync.dma_start(out=waT_hi[:], in_=wah_ext[:])
            nc.scalar.dma_start(out=waT_lo[:], in_=wal_ext[:])
            wbT_hi = wpool.tile([128, 2, COUT], bf16)
            wbT_lo = wpool.tile([128, 2, COUT], bf16)
            nc.sync.dma_start(out=wbT_hi[:], in_=wbh_ext[:])
            nc.sync.dma_start(out=wbT_lo[:], in_=wbl_ext[:])
            ba_sb = wpool.tile([128, 2], f32)
            nc.sync.dma_start(
                out=ba_sb[:], in_=ba_ext.ap().rearrange("(m p) -> p m", p=128)
            )
            bb_sb = wpool.tile([COUT, 1], f32)
            nc.sync.dma_start(out=bb_sb[:], in_=bb_ext[:])

            # padded input tiles: only the 1-px border needs zeroing (the
            # interior is DMA-overwritten every sample) -- thin strided
            # memsets keep the first x-load off the critical path
            x_ph = xpool.tile([128, 2, PPIX], bf16)
            x_pl = xpool.tile([128, 2, PPIX], bf16)
            for xt in (x_ph, x_pl):
                for k in range(2):
                    v = xt[:, k, :].rearrange("p (hh ww) -> p hh ww", hh=HP)
                    nc.vector.memset(v[:, 0, :], 0.0)       # top row
                    nc.vector.memset(v[:, HP - 1, :], 0.0)  # bottom row
                    nc.vector.memset(v[:, :, 0], 0.0)       # left col
                    nc.vector.memset(v[:, :, WP - 1], 0.0)  # right col

            h_hi = hpool.tile([128, 2, PIX], bf16)
            h_lo = hpool.tile([128, 2, PIX], bf16)
            logits_cm = cmpool.tile([COUT, PIX], f32)
            s_row = cmpool.tile([1, PIX], f32)
            eh_cm = cmpool.tile([COUT, PIX], bf16)
            el_cm = cmpool.tile([COUT, PIX], bf16)

            for b in range(B_PER_CORE):
                work_cm = wkpool.tile([COUT, PIX], f32, tag="work")
                # load x hi/lo into padded interiors; split by row-half across
                # the three DMA queues so the first conv taps (k=0) start early
                xdma = (nc.gpsimd, nc.sync, nc.scalar)
                i = 0
                for xt, xe in ((x_ph, xh_ext), (x_pl, xl_ext)):
                    for k in range(2):
                        for r0 in (0, H // 2):
                            xdma[i % 3].dma_start(
                                out=xt[:, k, :].rearrange(
                                    "p (hh ww) -> p hh ww", hh=HP
                                )[:, 1 + r0 : 1 + r0 + H // 2, 1 : 1 + W],
                                in_=xe.ap()[b].rearrange(
                                    "(k p) hh ww -> k p hh ww", p=128
                                )[k, :, r0 : r0 + H // 2],
                            )
                            i += 1
                # ---- conv-a (3x3) + BN + ReLU ----
                for m in range(2):
                    for tl in range(NTILE):
                        pa = pspool.tile([128, TILE_PIX], f32, tag="mm")
                        y0 = tl * TROWS
                        n_mm = 0
                        # pass-major order: hi*x_hi taps first so the first
                        # matmuls only depend on the x_hi DMAs
                        for wT, xt in (
                            (waT_hi, x_ph),
                            (waT_hi, x_pl),
                            (waT_lo, x_ph),
                        ):
                            for k in range(2):
                                for t in range(9):
                                    dy, dx = t // 3, t % 3
                                    rhs = xt[:, k, :].rearrange(
                                        "p (hh ww) -> p hh ww", hh=HP
                                    )[:, y0 + dy : y0 + dy + TROWS, dx : dx + W]
                                    nc.tensor.matmul(
                                        pa[:],
                                        wT[:, k * 9 + t, m * 128 : (m + 1) * 128],
                                        rhs,
                                        start=(n_mm == 0),
                                        stop=(n_mm == 53),
                                    )
                                    n_mm += 1
                        hs = wkpool.tile([128, TILE_PIX], f32, tag="hscr")
                        hs2 = wkpool.tile([128, TILE_PIX], f32, tag="hscr2")
                        sl = slice(tl * TILE_PIX, (tl + 1) * TILE_PIX)
                        nc.scalar.activation(
                            hs[:],
                            pa[:],
                            mybir.ActivationFunctionType.Relu,
                            bias=ba_sb[:, m : m + 1],
                            scale=1.0,
                        )
                        nc.scalar.copy(h_hi[:, m, sl], hs[:])
                        nc.vector.tensor_copy(hs2[:], h_hi[:, m, sl])
                        nc.vector.tensor_sub(hs2[:], hs[:], hs2[:])
                        nc.vector.tensor_copy(h_lo[:, m, sl], hs2[:])

                # ---- conv-b (1x1) + BN  (channel-major) ----
                for tl in range(NTILE):
                    pb = pspool.tile([COUT, TILE_PIX], f32, tag="mm")
                    n_mm = 0
                    for k in range(2):
                        hsl = slice(tl * TILE_PIX, (tl + 1) * TILE_PIX)
                        for lhs, rhs in (
                            (wbT_hi[:, k, :], h_hi[:, k, hsl]),
                            (wbT_hi[:, k, :], h_lo[:, k, hsl]),
                            (wbT_lo[:, k, :], h_hi[:, k, hsl]),
                        ):
                            nc.tensor.matmul(
                                pb[:], lhs, rhs, start=(n_mm == 0), stop=(n_mm == 5)
                            )
                            n_mm += 1
                    nc.vector.tensor_scalar_add(
                        logits_cm[:, tl * TILE_PIX : (tl + 1) * TILE_PIX],
                        pb[:],
                        bb_sb[:],
                    )
                nc.sync.dma_start(
                    out=logits_out.ap()[b].rearrange("c hh ww -> c (hh ww)"),
                    in_=logits_cm[:],
                )

                # ---- softmax over channel dim ----
                for tl in range(NTILE):
                    psum_s = ps1pool.tile([1, TILE_PIX], f32, tag="sm")
                    sl = slice(tl * TILE_PIX, (tl + 1) * TILE_PIX)
                    nc.scalar.activation(
                        work_cm[:, sl],
                        logits_cm[:, sl],
                        mybir.ActivationFunctionType.Exp,
                    )
                    nc.scalar.copy(eh_cm[:, sl], work_cm[:, sl])
                    nc.vector.tensor_sub(el_cm[:, sl], work_cm[:, sl], eh_cm[:, sl])
                    nc.tensor.matmul(
                        psum_s[:], ones_bf[:], eh_cm[:, sl], start=True, stop=False
                    )
                    nc.tensor.matmul(
                        psum_s[:], ones_bf[:], el_cm[:, sl], start=False, stop=True
                    )
                    nc.vector.tensor_copy(s_row[:, sl], psum_s[:])
                nc.vector.reciprocal(s_row[:], s_row[:])
                # broadcast 1/S to all channel partitions: bounce via DRAM with a
                # 0-stride read on the DRAM side; logits_cm is dead after its
                # output DMA, so reuse it as the broadcast target
                nc.sync.dma_start(out=s_dram.ap()[b][None, :], in_=s_row[:])
                nc.sync.dma_start(
                    out=logits_cm[:],
                    in_=s_dram.ap()[b][None, :].broadcast_to([COUT, PIX]),
                )
                nc.vector.tensor_mul(work_cm[:], work_cm[:], logits_cm[:])

                # ---- heat output: pixel shuffle via DMA access pattern ----
                # heat[b, hc*8+r, wc*8+cc] = prob[r*8+cc, hc*80+wc]
                dma_engines = (nc.sync, nc.scalar, nc.gpsimd)
                for r in range(GRID):
                    for cc in range(GRID):
                        c = r * GRID + cc
                        dma_engines[c % len(dma_engines)].dma_start(
                            out=heat_out.ap()[b].rearrange(
                                "(hc r) (wc cc) -> r cc hc wc", r=GRID, cc=GRID
                            )[r, cc],
                            in_=work_cm[c : c + 1, :].rearrange(
                                "c (hc wc) -> c hc wc", hc=H
                            ),
                        )

    nc.finalize()
    return nc


def _get_nc():
    if "nc" not in _CACHED:
        _CACHED["nc"] = _build_nc()
    return _CACHED["nc"]


def _host_nms(heat):
    """Exact replication of reference _box_nms (vectorized, Jacobi to fixpoint)."""
    B = heat.shape[0]
    flat = heat.reshape(B, -1)
    # top-1024 sorted desc, ties by index asc (matches jax top_k)
    idx = np.argsort(-flat, axis=1, kind="stable")[:, :NMS_CAND]
    scores = np.take_along_axis(flat, idx, axis=1)
    ys = (idx // HW_).astype(np.float32)
    xs = (idx % HW_).astype(np.float32)
    heat_nms = np.zeros_like(flat)
    for b in range(B):
        dy = np.abs(ys[b][:, None] - ys[b][None, :])
        dx = np.abs(xs[b][:, None] - xs[b][None, :])
        inter = np.maximum(NMS_SIZE - dy, 0.0) * np.maximum(NMS_SIZE - dx, 0.0)
        iou = inter / (2.0 * NMS_SIZE * NMS_SIZE - inter)
        overlap = iou > IOU_TH
        valid = scores[b] > MIN_PROB
        np.fill_diagonal(overlap, False)
        ov_ut = np.triu(overlap, 1)
        keep = valid.copy()
        for _ in range(NMS_CAND + 1):  # Jacobi fixpoint == greedy result;
            supp = ov_ut[keep].any(axis=0)  # converges in <= chain depth iters
            newkeep = valid & ~supp
            if (newkeep == keep).all():
                break
            keep = newkeep
        rank = np.cumsum(keep)
        keep = keep & (rank <= TOP_K)
        kept = np.where(keep, scores[b], 0.0).astype(np.float32)
        heat_nms[b, idx[b]] = kept
    return heat_nms.reshape(B, HH, HW_)


def kernel(x, Wa, ba, ga, bta, ma, va, Wb, bb, gb, btb, mb, vb):
    from concourse.bass_utils import run_bass_kernel_spmd

    import ml_dtypes

    x = np.ascontiguousarray(np.asarray(x, dtype=np.float32))
    x_hi = x.astype(ml_dtypes.bfloat16)
    x_lo = (x - x_hi.astype(np.float32)).astype(ml_dtypes.bfloat16)
    nc = _get_nc()

    # fold BN params on host (cheap per-channel math, not data-dependent)
    def bn_fold(g, v, m_, bt, bconv):
        g = np.asarray(g, np.float32)
        v = np.asarray(v, np.float32)
        m_ = np.asarray(m_, np.float32)
        bt = np.asarray(bt, np.float32)
        bconv = np.asarray(bconv, np.float32)
        scale = (g * (1.0 / np.sqrt(v + np.float32(EPS)))).astype(np.float32)
        bias = ((bconv - m_) * scale + bt).astype(np.float32)
        return scale, bias

    scale_a, bias_a = bn_fold(ga, va, ma, bta, ba)
    scale_b, bias_b = bn_fold(gb, vb, mb, btb, bb)

    # fold BN scale, transpose to [ci, (k,t), co] lhsT layout, split bf16 hi/lo
    WaS = np.asarray(Wa, np.float32).reshape(CMID, CIN, 9) * scale_a[:, None, None]
    WaT = WaS.transpose(1, 2, 0).reshape(2, 128, 9, CMID)  # [k, ci_p, t, co]
    WaT = np.ascontiguousarray(WaT.transpose(1, 0, 2, 3).reshape(128, 18, CMID))
    WaT_hi = WaT.astype(ml_dtypes.bfloat16)
    WaT_lo = (WaT - WaT_hi.astype(np.float32)).astype(ml_dtypes.bfloat16)
    WbS = np.asarray(Wb, np.float32).reshape(COUT, CMID) * scale_b[:, None]
    WbT = np.ascontiguousarray(
        WbS.transpose(1, 0).reshape(2, 128, COUT).transpose(1, 0, 2)
    )
    WbT_hi = WbT.astype(ml_dtypes.bfloat16)
    WbT_lo = (WbT - WbT_hi.astype(np.float32)).astype(ml_dtypes.bfloat16)

    ones128 = np.ones((128, 128), np.float32)

    in_maps = []
    for c in range(8):
        in_maps.append(
            {
                "x_hi": x_hi[c * B_PER_CORE : (c + 1) * B_PER_CORE],
                "x_lo": x_lo[c * B_PER_CORE : (c + 1) * B_PER_CORE],
                "waT_hi": WaT_hi,
                "waT_lo": WaT_lo,
                "wbT_hi": WbT_hi,
                "wbT_lo": WbT_lo,
                "bias_a": bias_a,
                "bias_b": bias_b.reshape(COUT, 1),
                "ones128": ones128,
            }
        )

    _CACHED["last_in_maps"] = in_maps
    res = run_bass_kernel_spmd(nc, in_maps, core_ids=list(range(8)))
    logits = np.concatenate([r["logits"] for r in res.results], axis=0)
    heat = np.concatenate([r["heat"] for r in res.results], axis=0)

    heat_nms = _host_nms(heat)
    pred = (heat_nms >= MIN_PROB).astype(np.int32)
    return logits, heat, heat_nms, pred
